# revision 74
# baseline (speedup 1.0000x reference)
"""Trainium2 Bass kernel for MatchingLayerL2:
   out = log_softmax(-sqrt(||x_i - y_j||^2) / std_j, axis=1)

x: [4096, 128] f32, y: [32768, 128] f32, std: [32768] f32 -> out [4096, 32768] f32.

Strategy: shard rows of x across 8 cores (512 rows each); y/std replicated.
Host prepares device inputs (layout/dtype prep only, O((N+M)D) work):
  yhatT = (y * r2[:,None]).T as bf16 [128, M]   (r2 = 1/std^2)
  xT    = (-2 x_c).T as bf16 [128, 512]
  corr rows (rank-2 term a_i*r2_j + bhat_j in hi/lo bf16 splits, K=5):
    cl = [a_hi; a_lo; a_hi; 1; 1]  [5, 512]
    cr = [r2_hi; r2_hi; r2_lo; bhat_hi; bhat_lo]  [5, M]
Device per core (512 rows = 4 row-blocks of 128):
  q = xT.T @ yhatT + cl.T @ cr   (PSUM f32, = r2_j * dist2_ij)
  s = sqrt(q)  fp16, unit = 1024 cols: first 7 units of each block on ACT
      Sqrt, the rest via DVE copy PSUM->SBUF fp16 + GPSIMD tensor_tensor
      pow 0.5 (GPSIMD cannot read PSUM; sqrt/exp exist only on ACT/Pool)
  S_i = sum_j exp(-s)  (ACT Exp + accum, fp8 scratch out; per-8192 instrs)
  out = -s - ln(S)     (DVE tensor_scalar into og staging, fp16) -> DMA
ACT runs Sqrt in one batch per block and Exp/Identity/Ln in another
(2 table loads/block); the last group's exp + Ln defer into the next
block so ACT keeps busy while Pool finishes the block's tail.
"""

import os
import sys

sys.path.insert(0, "/root/.axon_site/_ro/trn_rl_repo")

import numpy as np
import ml_dtypes
from contextlib import ExitStack

import concourse.bass as bass
from concourse import bacc
import concourse.tile as tile
from concourse.tile import add_dep_helper
from concourse import mybir
from concourse.bass_utils import run_bass_kernel_spmd

F32 = mybir.dt.float32
BF16 = mybir.dt.bfloat16
FP16 = mybir.dt.float16
FP8 = mybir.dt.float8e4
AF = mybir.ActivationFunctionType
ALU = mybir.AluOpType
AX = mybir.AxisListType

N_CORES = 8
D = 128
P = 128
UNIT = 1024           # PSUM ring unit (2 banks f32); 4-deep pipeline
GROUP = 8192          # columns per exp instruction / s sub-tile
HALFG = 4096          # final/og/out-store granularity
NA_BLOCK = 7          # leading units of each block handled by ACT Sqrt
BF = ml_dtypes.bfloat16


def build_nc(rows, M):
    NB = rows // P            # 4 row blocks of 128
    NG = M // GROUP           # 4 groups per block
    NU = GROUP // UNIT        # 8 units per group

    nc = bacc.Bacc("TRN2", target_bir_lowering=False, debug=False, num_swdge_queues=4)
    yT_d = nc.declare_dram_parameter("yT", [P, M], BF16, isOutput=False)
    xT_d = nc.declare_dram_parameter("xT", [P, rows], BF16, isOutput=False)
    cr_d = nc.declare_dram_parameter("cr", [5, M], BF16, isOutput=False)
    cl_d = nc.declare_dram_parameter("cl", [5, rows], BF16, isOutput=False)
    out_d = nc.declare_dram_parameter("out", [rows, M], FP16, isOutput=True)

    try:
        from concourse.hw_specs import get_activation_tables

        tabs = list(get_activation_tables(nc.m.arch).values())
        SQRT_SET = next(i for i, s in enumerate(tabs) if AF.Sqrt in s)
        EXPLN_SET = next(
            i for i, s in enumerate(tabs)
            if AF.Exp in s and AF.Ln in s and AF.Identity in s
        )
    except Exception:
        SQRT_SET, EXPLN_SET = 3, 6

    # The tile scheduler reorders instructions; chain each compute engine's
    # stream (sync=False ordering hints) so the balanced interleave survives.
    prev_inst = {}

    def chain(engine, binst):
        p = prev_inst.get(engine)
        if p is not None:
            add_dep_helper(binst.ins, p.ins, sync=False, reason=f"{engine} order")
        prev_inst[engine] = binst
        return binst

    def act(*a, **k):
        return chain("act", nc.scalar.activation(*a, **k))

    cur_table = [None]

    def ensure_table(set_id):
        if cur_table[0] == set_id:
            return
        cur_table[0] = set_id
        inst = mybir.InstLoadActFuncSet(
            name=nc.get_next_instruction_name(), ins=[], outs=[],
            act_func_set_id=set_id,
        )
        chain("act", nc.scalar.add_instruction(inst))

    # per-block unit schedule: ACT-sqrt units only in groups 0-1 (later
    # groups' s-tile slots aren't free yet at phase-1 time); Pool load per
    # group is [4,5,8,8] — the heavy tail carries Pool across the block
    # boundary while PE runs the next block's phase-1 units.
    ACT_UNITS = {0: (0, 1, 2, 3), 1: (0, 1, 2), 2: (), 3: ()}

    with tile.TileContext(nc) as tc, ExitStack() as ctx:
        pool = lambda name, bufs, space="SBUF": ctx.enter_context(
            tc.tile_pool(name=name, bufs=bufs, space=space)
        )
        const_p = pool("const", 1)
        s_p = pool("s", 5)
        es_p = pool("es", 1)
        cr_p = pool("cr", 3)
        crA_p = pool("crA", 4)
        og_p = pool("og", 4)
        scal_p = pool("scal", 8)
        mm_ps = pool("mmps", 4, space="PSUM")   # 4 x [128, 1024] f32 = 8 banks

        # resident inputs (yT pieces loaded just-in-time during block 0)
        xT = const_p.tile([P, rows], BF16)
        nc.sync.dma_start(out=xT[:], in_=xT_d[:, :])
        cl = const_p.tile([5, rows], BF16)
        nc.sync.dma_start(out=cl[:], in_=cl_d[:, :])
        half = const_p.tile([P, UNIT], FP16)
        chain("dve", nc.vector.memset(half[:], 0.5))
        yT = const_p.tile([P, M], BF16)

        pending = []  # (s_tile, lnS, b, g, q) finals awaiting emission

        OGW = 2048  # final/og/out-store granularity

        def emit_final(s_t, lnS, b, g, q):
            # og staging (not in-place) so the s tile's pool slot frees on
            # this read rather than on the out-store's completion
            og = og_p.tile([P, OGW], FP16)
            chain(
                "dve",
                nc.vector.tensor_scalar(
                    og[:], s_t[:, q * OGW : (q + 1) * OGW], -1.0,
                    lnS[:, 0:1], op0=ALU.mult, op1=ALU.subtract
                ),
            )
            j0 = g * GROUP + q * OGW
            nc.sync.dma_start(
                out=out_d[b * P : (b + 1) * P, j0 : j0 + OGW], in_=og[:]
            )

        def emit_exp(s_t, part, g):
            ensure_table(EXPLN_SET)
            es = es_p.tile([P, GROUP], FP8)
            act(es[:], s_t[:], AF.Exp, scale=-1.0, accum_out=part[:, g : g + 1])

        def make_tail(b, part, s_tiles):
            def tail():
                # last group's exp split in two so Ln lands earlier
                ensure_table(EXPLN_SET)
                s_t = s_tiles[NG - 1]
                es = es_p.tile([P, GROUP], FP8)
                act(es[:, 0:HALFG], s_t[:, 0:HALFG], AF.Exp, scale=-1.0,
                    accum_out=part[:, NG - 1 : NG])
                act(es[:, HALFG:], s_t[:, HALFG:], AF.Exp, scale=-1.0,
                    accum_out=part[:, NG : NG + 1])
                # partial sum + ln on ACT itself (Identity/Ln share the Exp
                # table; on DVE this would stall its in-order queue)
                junk = scal_p.tile([P, NG + 1], F32, tag="junk")
                S = scal_p.tile([P, 1], F32, tag="S")
                act(junk[:], part[:], AF.Identity, accum_out=S[:])
                lnS = scal_p.tile([P, 1], F32, tag="lnS")
                act(lnS[:], S[:], AF.Ln)
                for g in range(NG):
                    for q in range(GROUP // OGW):
                        pending.append((s_tiles[g], lnS, b, g, q))
            return tail

        # corr-row tiles for pool-path units, prefetched one group ahead so
        # their small DMAs dispatch before bulky out-stores on the DMA FIFO
        cr_tiles = {}

        CRW = 4096

        def prefetch_cr(b, g):
            if b >= NB:
                return
            for hf in range(GROUP // CRW):
                j0 = g * GROUP + hf * CRW
                t = cr_p.tile([5, CRW], BF16)
                nc.sync.dma_start(out=t[:], in_=cr_d[:, j0 : j0 + CRW])
                cr_tiles[(b, g, hf)] = t

        def emit_unit_mm(b, g, u, s_t, is_act):
            j0 = g * GROUP + u * UNIT
            if b == 0:
                nc.sync.dma_start(
                    out=yT[:, j0 : j0 + UNIT], in_=yT_d[:, j0 : j0 + UNIT]
                )
            if is_act:
                cr_t = crA_p.tile([5, UNIT], BF16)
                nc.sync.dma_start(out=cr_t[:], in_=cr_d[:, j0 : j0 + UNIT])
                co = 0
            else:
                cr_t = cr_tiles[(b, g, u // (CRW // UNIT))]
                co = (u % (CRW // UNIT)) * UNIT
            mm = mm_ps.tile([P, UNIT], F32)
            for q in range(UNIT // 512):
                nc.tensor.matmul(
                    mm[:, 512 * q : 512 * (q + 1)],
                    xT[:, b * P : (b + 1) * P],
                    yT[:, j0 + 512 * q : j0 + 512 * (q + 1)],
                    start=True,
                    stop=False,
                )
            for q in range(UNIT // 512):
                nc.tensor.matmul(
                    mm[:, 512 * q : 512 * (q + 1)],
                    cl[:, b * P : (b + 1) * P],
                    cr_t[:, co + 512 * q : co + 512 * (q + 1)],
                    start=False,
                    stop=True,
                )
            return mm, s_t[:, u * UNIT : (u + 1) * UNIT]

        def emit_unit(b, g, u, s_t, is_act):
            mm, sl = emit_unit_mm(b, g, u, s_t, is_act)
            if is_act:
                ensure_table(SQRT_SET)
                act(sl, mm[:], AF.Sqrt)
            else:
                chain("dve", nc.vector.tensor_copy(sl, mm[:]))
                chain("pool", nc.gpsimd.tensor_tensor(sl, sl, half[:], op=ALU.pow))

        prev_tail = None
        prefetch_cr(0, 0)
        s_map = {}

        def get_s(b, g):
            if (b, g) not in s_map:
                s_map[(b, g)] = s_p.tile([P, GROUP], FP16, name="s_t")
            return s_map[(b, g)]

        for b in range(NB):
            part = scal_p.tile([P, NG + 1], F32, tag="part")
            s_tiles = [get_s(b, g) for g in range(NG)]

            def pop_finals(n):
                for _ in range(n):
                    if pending:
                        emit_final(*pending.pop(0))

            # g0's pool units first: they keep Pool/DVE/PE fed through the
            # previous block's ACT tail (the PSUM ring cycles through them)
            prefetch_cr(b, 1)
            if b == 0:
                for u in ACT_UNITS[0]:
                    emit_unit(0, 0, u, s_tiles[0], True)
            for u in range(NU):
                if u not in ACT_UNITS[0]:
                    emit_unit(b, 0, u, s_tiles[0], False)
            if prev_tail is not None:
                prev_tail()
                prev_tail = None
            pop_finals(6)
            # remaining ACT sqrt units (g1) interleaved 1:1 with g1's pool
            # units so both consumers drain the PSUM ring
            prefetch_cr(b, 2)
            A_G1 = [(1, u) for u in ACT_UNITS[1]]
            g1_pool = [u for u in range(NU) if u not in ACT_UNITS[1]]
            seq = []
            for i in range(max(len(A_G1), len(g1_pool))):
                if i < len(A_G1):
                    seq.append((1, A_G1[i][1], True))
                if i < len(g1_pool):
                    seq.append((1, g1_pool[i], False))
            for i, (g, u, is_act) in enumerate(seq):
                emit_unit(b, g, u, s_tiles[g], is_act)
                if i == 4:
                    pop_finals(2)
            emit_exp(s_tiles[0], part, 0)
            # remaining pool-path groups
            for g in range(2, NG):
                if g + 1 < NG:
                    prefetch_cr(b, g + 1)
                else:
                    prefetch_cr(b + 1, 0)
                pop_finals(2)
                np_done = 0
                for u in range(NU):
                    if u not in ACT_UNITS[g]:
                        emit_unit(b, g, u, s_tiles[g], False)
                        np_done += 1
                        if np_done == 4:
                            pop_finals(2)
                emit_exp(s_tiles[g - 1], part, g - 1)
            # seam: next block's g0 sqrt units run here, filling ACT's wait
            # for this block's tail group and feeding PE/PSUM over the edge
            if b + 1 < NB:
                for u in ACT_UNITS[0]:
                    emit_unit(b + 1, 0, u, get_s(b + 1, 0), True)
            prev_tail = make_tail(b, part, s_tiles)
        prev_tail()
        while pending:
            emit_final(*pending.pop(0))

    nc.finalize()
    return nc


_NC_CACHE = {}


def _get_nc(rows, M):
    key = (rows, M)
    if key not in _NC_CACHE:
        _NC_CACHE[key] = build_nc(rows, M)
    return _NC_CACHE[key]


def _hi_lo(v32):
    hi = v32.astype(BF)
    lo = (v32 - hi.astype(np.float32)).astype(BF)
    return hi, lo


def kernel(x: np.ndarray, y: np.ndarray, std: np.ndarray) -> np.ndarray:
    x = np.ascontiguousarray(x, dtype=np.float32)
    y = np.ascontiguousarray(y, dtype=np.float32)
    std = np.ascontiguousarray(std, dtype=np.float32)
    N, M = x.shape[0], y.shape[0]
    rows = N // N_CORES

    r2 = (1.0 / (std.astype(np.float64) ** 2)).astype(np.float32)
    yhatT = np.ascontiguousarray((y.T * r2[None, :]).astype(BF))
    bhat = ((y.astype(np.float64) ** 2).sum(axis=1) * r2.astype(np.float64)).astype(
        np.float32
    )
    r2_hi, r2_lo = _hi_lo(r2)
    b_hi, b_lo = _hi_lo(bhat)
    cr = np.ascontiguousarray(np.stack([r2_hi, r2_hi, r2_lo, b_hi, b_lo]))

    a = (x.astype(np.float64) ** 2).sum(axis=1).astype(np.float32)
    a_hi, a_lo = _hi_lo(a)
    ones = np.ones_like(a_hi)
    xT_all = np.ascontiguousarray((-2.0 * x.T).astype(BF))

    in_maps = []
    for c in range(N_CORES):
        sl = slice(c * rows, (c + 1) * rows)
        cl = np.ascontiguousarray(
            np.stack([a_hi[sl], a_lo[sl], a_hi[sl], ones[sl], ones[sl]])
        )
        in_maps.append(
            {
                "yT": yhatT,
                "xT": np.ascontiguousarray(xT_all[:, sl]),
                "cr": cr,
                "cl": cl,
            }
        )

    nc = _get_nc(rows, M)
    trace = bool(int(os.environ.get("KERNEL_TRACE", "0")))
    res = run_bass_kernel_spmd(
        nc, in_maps, core_ids=list(range(N_CORES)), trace=trace
    )
    global LAST_RESULT
    LAST_RESULT = res
    return np.concatenate(
        [res.results[c]["out"].astype(np.float32) for c in range(N_CORES)], axis=0
    )


LAST_RESULT = None


# revision 75
# speedup vs baseline: 1.0012x; 1.0012x over previous
"""Trainium2 Bass kernel for MatchingLayerL2:
   out = log_softmax(-sqrt(||x_i - y_j||^2) / std_j, axis=1)

x: [4096, 128] f32, y: [32768, 128] f32, std: [32768] f32 -> out [4096, 32768] f32.

Strategy: shard rows of x across 8 cores (512 rows each); y/std replicated.
Host prepares device inputs (layout/dtype prep only, O((N+M)D) work):
  yhatT = (y * r2[:,None]).T as bf16 [128, M]   (r2 = 1/std^2)
  xT    = (-2 x_c).T as bf16 [128, 512]
  corr rows (rank-2 term a_i*r2_j + bhat_j in hi/lo bf16 splits, K=5):
    cl = [a_hi; a_lo; a_hi; 1; 1]  [5, 512]
    cr = [r2_hi; r2_hi; r2_lo; bhat_hi; bhat_lo]  [5, M]
Device per core (512 rows = 4 row-blocks of 128):
  q = xT.T @ yhatT + cl.T @ cr   (PSUM f32, = r2_j * dist2_ij)
  s = sqrt(q)  fp16, unit = 1024 cols: first 7 units of each block on ACT
      Sqrt, the rest via DVE copy PSUM->SBUF fp16 + GPSIMD tensor_tensor
      pow 0.5 (GPSIMD cannot read PSUM; sqrt/exp exist only on ACT/Pool)
  S_i = sum_j exp(-s)  (ACT Exp + accum, fp8 scratch out; per-8192 instrs)
  out = -s - ln(S)     (DVE tensor_scalar into og staging, fp16) -> DMA
ACT runs Sqrt in one batch per block and Exp/Identity/Ln in another
(2 table loads/block); the last group's exp + Ln defer into the next
block so ACT keeps busy while Pool finishes the block's tail.
"""

import os
import sys

sys.path.insert(0, "/root/.axon_site/_ro/trn_rl_repo")

import numpy as np
import ml_dtypes
from contextlib import ExitStack

import concourse.bass as bass
from concourse import bacc
import concourse.tile as tile
from concourse.tile import add_dep_helper
from concourse import mybir
from concourse.bass_utils import run_bass_kernel_spmd

F32 = mybir.dt.float32
BF16 = mybir.dt.bfloat16
FP16 = mybir.dt.float16
FP8 = mybir.dt.float8e4
AF = mybir.ActivationFunctionType
ALU = mybir.AluOpType
AX = mybir.AxisListType

N_CORES = 8
D = 128
P = 128
UNIT = 1024           # PSUM ring unit (2 banks f32); 4-deep pipeline
GROUP = 8192          # columns per exp instruction / s sub-tile
HALFG = 4096          # final/og/out-store granularity
NA_BLOCK = 7          # leading units of each block handled by ACT Sqrt
BF = ml_dtypes.bfloat16


def build_nc(rows, M):
    NB = rows // P            # 4 row blocks of 128
    NG = M // GROUP           # 4 groups per block
    NU = GROUP // UNIT        # 8 units per group

    nc = bacc.Bacc("TRN2", target_bir_lowering=False, debug=False, num_swdge_queues=4)
    yT_d = nc.declare_dram_parameter("yT", [P, M], BF16, isOutput=False)
    xT_d = nc.declare_dram_parameter("xT", [P, rows], BF16, isOutput=False)
    cr_d = nc.declare_dram_parameter("cr", [5, M], BF16, isOutput=False)
    cl_d = nc.declare_dram_parameter("cl", [5, rows], BF16, isOutput=False)
    out_d = nc.declare_dram_parameter("out", [rows, M], FP16, isOutput=True)

    try:
        from concourse.hw_specs import get_activation_tables

        tabs = list(get_activation_tables(nc.m.arch).values())
        SQRT_SET = next(i for i, s in enumerate(tabs) if AF.Sqrt in s)
        EXPLN_SET = next(
            i for i, s in enumerate(tabs)
            if AF.Exp in s and AF.Ln in s and AF.Identity in s
        )
    except Exception:
        SQRT_SET, EXPLN_SET = 3, 6

    # The tile scheduler reorders instructions; chain each compute engine's
    # stream (sync=False ordering hints) so the balanced interleave survives.
    prev_inst = {}

    def chain(engine, binst):
        p = prev_inst.get(engine)
        if p is not None:
            add_dep_helper(binst.ins, p.ins, sync=False, reason=f"{engine} order")
        prev_inst[engine] = binst
        return binst

    def act(*a, **k):
        return chain("act", nc.scalar.activation(*a, **k))

    cur_table = [None]

    def ensure_table(set_id):
        if cur_table[0] == set_id:
            return
        cur_table[0] = set_id
        inst = mybir.InstLoadActFuncSet(
            name=nc.get_next_instruction_name(), ins=[], outs=[],
            act_func_set_id=set_id,
        )
        chain("act", nc.scalar.add_instruction(inst))

    # per-block unit schedule: ACT-sqrt units only in groups 0-1 (later
    # groups' s-tile slots aren't free yet at phase-1 time); Pool load per
    # group is [4,5,8,8] — the heavy tail carries Pool across the block
    # boundary while PE runs the next block's phase-1 units.
    ACT_UNITS = {0: (0, 1, 2, 3), 1: (0, 1, 2), 2: (), 3: ()}

    with tile.TileContext(nc) as tc, ExitStack() as ctx:
        pool = lambda name, bufs, space="SBUF": ctx.enter_context(
            tc.tile_pool(name=name, bufs=bufs, space=space)
        )
        const_p = pool("const", 1)
        s_p = pool("s", 5)
        es_p = pool("es", 1)
        cr_p = pool("cr", 3)
        crA_p = pool("crA", 4)
        og_p = pool("og", 4)
        scal_p = pool("scal", 8)
        mm_ps = pool("mmps", 4, space="PSUM")   # 4 x [128, 1024] f32 = 8 banks

        # resident inputs (yT pieces loaded just-in-time during block 0)
        xT = const_p.tile([P, rows], BF16)
        nc.sync.dma_start(out=xT[:], in_=xT_d[:, :])
        cl = const_p.tile([5, rows], BF16)
        nc.sync.dma_start(out=cl[:], in_=cl_d[:, :])
        half = const_p.tile([P, UNIT], FP16)
        chain("dve", nc.vector.memset(half[:], 0.5))
        yT = const_p.tile([P, M], BF16)

        pending = []  # (s_tile, lnS, b, g, q) finals awaiting emission

        OGW = 2048  # final/og/out-store granularity

        def emit_final(s_t, lnS, b, g, q):
            # og staging (not in-place) so the s tile's pool slot frees on
            # this read rather than on the out-store's completion
            og = og_p.tile([P, OGW], FP16)
            chain(
                "dve",
                nc.vector.tensor_scalar(
                    og[:], s_t[:, q * OGW : (q + 1) * OGW], -1.0,
                    lnS[:, 0:1], op0=ALU.mult, op1=ALU.subtract
                ),
            )
            j0 = g * GROUP + q * OGW
            nc.sync.dma_start(
                out=out_d[b * P : (b + 1) * P, j0 : j0 + OGW], in_=og[:]
            )

        def emit_exp(s_t, part, g):
            ensure_table(EXPLN_SET)
            es = es_p.tile([P, GROUP], FP8)
            act(es[:], s_t[:], AF.Exp, scale=-1.0, accum_out=part[:, g : g + 1])

        def make_tail(b, part, s_tiles):
            def tail():
                # last group's exp split so Ln lands sooner after the final
                # pow; finer split for the last block, whose Ln gates the
                # end-of-kernel store drain
                ensure_table(EXPLN_SET)
                s_t = s_tiles[NG - 1]
                es = es_p.tile([P, GROUP], FP8)
                nsp = 4 if b == NB - 1 else 2
                w = GROUP // nsp
                for k in range(nsp):
                    act(es[:, k * w : (k + 1) * w], s_t[:, k * w : (k + 1) * w],
                        AF.Exp, scale=-1.0,
                        accum_out=part[:, NG - 1 + k : NG + k])
                # partial sum + ln on ACT itself (Identity/Ln share the Exp
                # table; on DVE this would stall its in-order queue); sum only
                # the columns this block wrote
                nc_cols = NG - 1 + nsp
                junk = scal_p.tile([P, NG + 3], F32, tag="junk")
                S = scal_p.tile([P, 1], F32, tag="S")
                act(junk[:, 0:nc_cols], part[:, 0:nc_cols], AF.Identity,
                    accum_out=S[:])
                lnS = scal_p.tile([P, 1], F32, tag="lnS")
                act(lnS[:], S[:], AF.Ln)
                for g in range(NG):
                    for q in range(GROUP // OGW):
                        pending.append((s_tiles[g], lnS, b, g, q))
            return tail

        # corr-row tiles for pool-path units, prefetched one group ahead so
        # their small DMAs dispatch before bulky out-stores on the DMA FIFO
        cr_tiles = {}

        CRW = 4096

        def prefetch_cr(b, g):
            if b >= NB:
                return
            for hf in range(GROUP // CRW):
                j0 = g * GROUP + hf * CRW
                t = cr_p.tile([5, CRW], BF16)
                nc.sync.dma_start(out=t[:], in_=cr_d[:, j0 : j0 + CRW])
                cr_tiles[(b, g, hf)] = t

        def emit_unit_mm(b, g, u, s_t, is_act):
            j0 = g * GROUP + u * UNIT
            if b == 0:
                nc.sync.dma_start(
                    out=yT[:, j0 : j0 + UNIT], in_=yT_d[:, j0 : j0 + UNIT]
                )
            if is_act:
                cr_t = crA_p.tile([5, UNIT], BF16)
                nc.sync.dma_start(out=cr_t[:], in_=cr_d[:, j0 : j0 + UNIT])
                co = 0
            else:
                cr_t = cr_tiles[(b, g, u // (CRW // UNIT))]
                co = (u % (CRW // UNIT)) * UNIT
            mm = mm_ps.tile([P, UNIT], F32)
            for q in range(UNIT // 512):
                nc.tensor.matmul(
                    mm[:, 512 * q : 512 * (q + 1)],
                    xT[:, b * P : (b + 1) * P],
                    yT[:, j0 + 512 * q : j0 + 512 * (q + 1)],
                    start=True,
                    stop=False,
                )
            for q in range(UNIT // 512):
                nc.tensor.matmul(
                    mm[:, 512 * q : 512 * (q + 1)],
                    cl[:, b * P : (b + 1) * P],
                    cr_t[:, co + 512 * q : co + 512 * (q + 1)],
                    start=False,
                    stop=True,
                )
            return mm, s_t[:, u * UNIT : (u + 1) * UNIT]

        def emit_unit(b, g, u, s_t, is_act):
            mm, sl = emit_unit_mm(b, g, u, s_t, is_act)
            if is_act:
                ensure_table(SQRT_SET)
                act(sl, mm[:], AF.Sqrt)
            else:
                chain("dve", nc.vector.tensor_copy(sl, mm[:]))
                chain("pool", nc.gpsimd.tensor_tensor(sl, sl, half[:], op=ALU.pow))

        prev_tail = None
        prefetch_cr(0, 0)
        s_map = {}

        def get_s(b, g):
            if (b, g) not in s_map:
                s_map[(b, g)] = s_p.tile([P, GROUP], FP16, name="s_t")
            return s_map[(b, g)]

        for b in range(NB):
            part = scal_p.tile([P, NG + 3], F32, tag="part")
            s_tiles = [get_s(b, g) for g in range(NG)]

            def pop_finals(n):
                for _ in range(n):
                    if pending:
                        emit_final(*pending.pop(0))

            # g0's pool units first: they keep Pool/DVE/PE fed through the
            # previous block's ACT tail (the PSUM ring cycles through them)
            prefetch_cr(b, 1)
            if b == 0:
                for u in ACT_UNITS[0]:
                    emit_unit(0, 0, u, s_tiles[0], True)
            for u in range(NU):
                if u not in ACT_UNITS[0]:
                    emit_unit(b, 0, u, s_tiles[0], False)
            if prev_tail is not None:
                prev_tail()
                prev_tail = None
            pop_finals(6)
            # remaining ACT sqrt units (g1) interleaved 1:1 with g1's pool
            # units so both consumers drain the PSUM ring
            prefetch_cr(b, 2)
            A_G1 = [(1, u) for u in ACT_UNITS[1]]
            g1_pool = [u for u in range(NU) if u not in ACT_UNITS[1]]
            seq = []
            for i in range(max(len(A_G1), len(g1_pool))):
                if i < len(A_G1):
                    seq.append((1, A_G1[i][1], True))
                if i < len(g1_pool):
                    seq.append((1, g1_pool[i], False))
            for i, (g, u, is_act) in enumerate(seq):
                emit_unit(b, g, u, s_tiles[g], is_act)
                if i == 4:
                    pop_finals(2)
            emit_exp(s_tiles[0], part, 0)
            # remaining pool-path groups
            for g in range(2, NG):
                if g + 1 < NG:
                    prefetch_cr(b, g + 1)
                else:
                    prefetch_cr(b + 1, 0)
                pop_finals(2)
                np_done = 0
                for u in range(NU):
                    if u not in ACT_UNITS[g]:
                        emit_unit(b, g, u, s_tiles[g], False)
                        np_done += 1
                        if np_done == 4:
                            pop_finals(2)
                emit_exp(s_tiles[g - 1], part, g - 1)
            # seam: next block's g0 sqrt units run here, filling ACT's wait
            # for this block's tail group and feeding PE/PSUM over the edge
            if b + 1 < NB:
                for u in ACT_UNITS[0]:
                    emit_unit(b + 1, 0, u, get_s(b + 1, 0), True)
            prev_tail = make_tail(b, part, s_tiles)
        prev_tail()
        while pending:
            emit_final(*pending.pop(0))

    nc.finalize()
    return nc


_NC_CACHE = {}


def _get_nc(rows, M):
    key = (rows, M)
    if key not in _NC_CACHE:
        _NC_CACHE[key] = build_nc(rows, M)
    return _NC_CACHE[key]


def _hi_lo(v32):
    hi = v32.astype(BF)
    lo = (v32 - hi.astype(np.float32)).astype(BF)
    return hi, lo


def kernel(x: np.ndarray, y: np.ndarray, std: np.ndarray) -> np.ndarray:
    x = np.ascontiguousarray(x, dtype=np.float32)
    y = np.ascontiguousarray(y, dtype=np.float32)
    std = np.ascontiguousarray(std, dtype=np.float32)
    N, M = x.shape[0], y.shape[0]
    rows = N // N_CORES

    r2 = (1.0 / (std.astype(np.float64) ** 2)).astype(np.float32)
    yhatT = np.ascontiguousarray((y.T * r2[None, :]).astype(BF))
    bhat = ((y.astype(np.float64) ** 2).sum(axis=1) * r2.astype(np.float64)).astype(
        np.float32
    )
    r2_hi, r2_lo = _hi_lo(r2)
    b_hi, b_lo = _hi_lo(bhat)
    cr = np.ascontiguousarray(np.stack([r2_hi, r2_hi, r2_lo, b_hi, b_lo]))

    a = (x.astype(np.float64) ** 2).sum(axis=1).astype(np.float32)
    a_hi, a_lo = _hi_lo(a)
    ones = np.ones_like(a_hi)
    xT_all = np.ascontiguousarray((-2.0 * x.T).astype(BF))

    in_maps = []
    for c in range(N_CORES):
        sl = slice(c * rows, (c + 1) * rows)
        cl = np.ascontiguousarray(
            np.stack([a_hi[sl], a_lo[sl], a_hi[sl], ones[sl], ones[sl]])
        )
        in_maps.append(
            {
                "yT": yhatT,
                "xT": np.ascontiguousarray(xT_all[:, sl]),
                "cr": cr,
                "cl": cl,
            }
        )

    nc = _get_nc(rows, M)
    trace = bool(int(os.environ.get("KERNEL_TRACE", "0")))
    res = run_bass_kernel_spmd(
        nc, in_maps, core_ids=list(range(N_CORES)), trace=trace
    )
    global LAST_RESULT
    LAST_RESULT = res
    return np.concatenate(
        [res.results[c]["out"].astype(np.float32) for c in range(N_CORES)], axis=0
    )


LAST_RESULT = None
